# revision 3
# baseline (speedup 1.0000x reference)
"""CoAttention kernel for Trainium2 (nn_CoAttention_77592879169836).

Full inputs in, full outputs out. Sharding: data-parallel over batch B=8,
one batch element per NeuronCore (8 cores), projection weights replicated.
No collectives needed.

Per-core math (x1,x2: [L,D], L=2048, D=1024, fp32):
  q = (x1 @ Wq^T + bq) / 32 ;  k = x2 @ Wk^T + bk
  v1 = x1 @ Wv^T + bv       ;  v2 = x2 @ Wv^T + bv
  s[l,m] = q[l]·k[m]                       (scale folded into q)
  Em = exp(s + (1-mask[m])*(-1e30))        (masked exponentials, bf16)
  c2[l] = sum_m Em[l,m]                    (exp accum_out)
  c1[m] = sum_l Em[l,m]                    (ones-column matmul)
  x1_mid[m,:] = (Em^T @ v1)[m,:] * mask[m]/(c1[m]+eps)
  x2_out = (EmT @ v2) / c2[:,None]         (EmT tiles via DMA transpose)
  x1_out = (EmT @ x1_mid) / c2[:,None]
All matmuls in bf16 with fp32 PSUM accumulation.
"""
import numpy as np
from contextlib import ExitStack

import concourse.bass as bass
import concourse.tile as tile
from concourse import bacc, mybir
from concourse.masks import make_identity

P = 128
B = 8
L = 2048          # L1 == L2
D = 1024
NT = L // P       # 16 row tiles
ND = D // P       # 8 feature chunks
NB = 512          # matmul moving-dim size
NEG = -1.0e30
BF = mybir.dt.bfloat16
F32 = mybir.dt.float32

_CACHE = {}


def _build(nc_L=L, nc_D=D):
    """Build + compile the single-core program (SPMD across 8 cores)."""
    L_, D_ = nc_L, nc_D
    NT_ = L_ // P
    ND_ = D_ // P
    NBL = min(NB, L_)          # l-block width for projections
    NLB = L_ // NBL            # number of l-blocks
    LT_B = NBL // P            # l-tiles per block
    NEH = max(1, D_ // NB)     # e halves (512 cols each)
    EH = D_ // NEH
    NMC = L_ // NB             # m chunks of 512 in scores

    nc = bacc.Bacc("TRN2", target_bir_lowering=False, debug=False)

    x1_in = nc.dram_tensor("x1", [L_, D_], F32, kind="ExternalInput").ap()
    x2_in = nc.dram_tensor("x2", [L_, D_], F32, kind="ExternalInput").ap()
    mask_in = nc.dram_tensor("mask", [L_], F32, kind="ExternalInput").ap()
    w_in = {}
    b_in = {}
    for nm in ("q", "k", "v"):
        w_in[nm] = nc.dram_tensor(f"W{nm}", [D_, D_], F32, kind="ExternalInput").ap()
        b_in[nm] = nc.dram_tensor(f"b{nm}", [D_], F32, kind="ExternalInput").ap()
    x1o_d = nc.dram_tensor("x1_out", [L_, D_], F32, kind="ExternalOutput").ap()
    x2o_d = nc.dram_tensor("x2_out", [L_, D_], F32, kind="ExternalOutput").ap()
    v1_d = nc.dram_tensor("v1_scr", [L_, D_], BF, kind="Internal").ap()

    scale = 1.0 / np.sqrt(np.float32(D_))

    with tile.TileContext(nc) as tc, ExitStack() as ctx:
        const = ctx.enter_context(tc.tile_pool(name="const", bufs=1))
        vpool = ctx.enter_context(tc.tile_pool(name="vpool", bufs=1))
        qk = ctx.enter_context(tc.tile_pool(name="qk", bufs=2))

        # ---------- constants ----------
        ident = const.tile([P, P], F32)
        make_identity(nc, ident[:])
        ones_row = const.tile([1, P], BF)      # K=1 lhsT of ones
        nc.any.memset(ones_row[:], 1.0)
        ones_col = const.tile([P, 1], BF)      # N=1 rhs of ones
        nc.any.memset(ones_col[:], 1.0)

        mrow_f = const.tile([1, L_], F32)
        nc.sync.dma_start(mrow_f[:], mask_in[None, :])
        negrow = const.tile([1, L_], BF)       # (mask-1)*1e30 -> 0 or -1e30
        nc.vector.tensor_scalar(
            out=negrow[:], in0=mrow_f[:], scalar1=1.0e30, scalar2=-1.0e30,
            op0=mybir.AluOpType.mult, op1=mybir.AluOpType.add)
        maskcol = const.tile([P, NT_], F32)
        nc.sync.dma_start(maskcol[:], mask_in.rearrange("(t p) -> p t", p=P))

        bcol = {}
        for nm in ("q", "k"):
            raw = const.tile([P, ND_], F32, tag=f"b{nm}raw", name=f"b{nm}raw")
            nc.sync.dma_start(raw[:], b_in[nm].rearrange("(c p) -> p c", p=P))
            if nm == "q":
                bcol[nm] = const.tile([P, ND_], F32, tag="bqs", name="bqs")
                nc.vector.tensor_scalar_mul(bcol[nm][:], raw[:], float(scale))
            else:
                bcol[nm] = raw
        bvrow_f = const.tile([1, D_], F32)
        nc.sync.dma_start(bvrow_f[:], b_in["v"][None, :])
        bvrow = const.tile([1, D_], BF)
        nc.vector.tensor_copy(bvrow[:], bvrow_f[:])

        c2r_sb = const.tile([P, NT_], F32)     # 1/c2 per l-tile

        # ---------- persistent big tiles ----------
        v2_sb = vpool.tile([P, NT_, D_], BF)   # v2, [m-part, mt, e]

        # qk pool: two slots shared by qT/kT then v1r/x1mid
        qT_sb = qk.tile([P, ND_, L_], BF, tag="qk")
        kT_sb = qk.tile([P, ND_, L_], BF, tag="qk")

        # =========== P0: transpose weights ===========
        with ExitStack() as pctx:
            wpool = pctx.enter_context(tc.tile_pool(name="wpool", bufs=2))
            stg = pctx.enter_context(tc.tile_pool(name="stg01", bufs=2))
            ps01 = pctx.enter_context(tc.tile_pool(name="ps01", bufs=4, space="PSUM"))

            def transpose_w(nm, scl):
                wT = wpool.tile([P, ND_, D_], BF, tag="w", name=f"w{nm}T")
                for ce in range(ND_):
                    wnat = stg.tile([P, D_], F32, tag="wnat")
                    nc.sync.dma_start(wnat[:], w_in[nm][ce * P:(ce + 1) * P, :])
                    for cd in range(ND_):
                        tp = ps01.tile([P, P], F32, tag="tp")
                        nc.tensor.transpose(tp[:], wnat[:, cd * P:(cd + 1) * P], ident[:])
                        dst = wT[:, cd, ce * P:(ce + 1) * P]
                        if scl is None:
                            nc.vector.tensor_copy(dst, tp[:])
                        else:
                            nc.vector.tensor_scalar_mul(dst, tp[:], scl)
                return wT

            wqT = transpose_w("q", float(scale))
            wvT = transpose_w("v", None)

            # =========== P1: projections ===========
            def x_pass(x_in, wT, bc, qT_dst, v_to_dram):
                for lb in range(NLB):
                    xT_blk = stg.tile([P, ND_, NBL], BF, tag="xTblk")
                    for li in range(LT_B):
                        lt = lb * LT_B + li
                        xstg = stg.tile([P, D_], F32, tag="xstg")
                        nc.sync.dma_start(xstg[:], x_in[lt * P:(lt + 1) * P, :])
                        for cd in range(ND_):
                            tp = ps01.tile([P, P], F32, tag="tp")
                            nc.tensor.transpose(
                                tp[:], xstg[:, cd * P:(cd + 1) * P], ident[:])
                            nc.vector.tensor_copy(
                                xT_blk[:, cd, li * P:(li + 1) * P], tp[:])
                    # qT / kT projection: out [e-chunk, l-block]
                    for ce in range(ND_):
                        qp = ps01.tile([P, NBL], F32, tag="proj")
                        for cd in range(ND_):
                            nc.tensor.matmul(
                                qp[:], wT[:, cd, ce * P:(ce + 1) * P],
                                xT_blk[:, cd, :],
                                start=(cd == 0), stop=(cd == ND_ - 1))
                        nc.vector.tensor_scalar_add(
                            qT_dst[:, ce, lb * NBL:(lb + 1) * NBL], qp[:],
                            bc[:, ce:ce + 1])
                    # v projection: out [l-tile, e]
                    for li in range(LT_B):
                        lt = lb * LT_B + li
                        vstg = stg.tile([P, D_], BF, tag="vstg")
                        for eh in range(NEH):
                            vp = ps01.tile([P, EH], F32, tag="proj")
                            for cd in range(ND_):
                                nc.tensor.matmul(
                                    vp[:], xT_blk[:, cd, li * P:(li + 1) * P],
                                    wvT[:, cd, eh * EH:(eh + 1) * EH],
                                    start=(cd == 0), stop=False)
                            nc.tensor.matmul(
                                vp[:], ones_row[:], bvrow[0:1, eh * EH:(eh + 1) * EH],
                                start=False, stop=True)
                            if v_to_dram:
                                nc.vector.tensor_copy(
                                    vstg[:, eh * EH:(eh + 1) * EH], vp[:])
                            else:
                                nc.vector.tensor_copy(
                                    v2_sb[:, lt, eh * EH:(eh + 1) * EH], vp[:])
                        if v_to_dram:
                            nc.sync.dma_start(v1_d[lt * P:(lt + 1) * P, :], vstg[:])

            x_pass(x1_in, wqT, bcol["q"], qT_sb, True)
            wkT = transpose_w("k", None)   # reuses wq's slot
            x_pass(x2_in, wkT, bcol["k"], kT_sb, False)

        # E pool opens only after P1 staging is released (SBUF budget)
        epool = ctx.enter_context(tc.tile_pool(name="epool", bufs=1))
        E_sb = epool.tile([P, NT_, L_], BF)    # masked exp(s), [l-part, lt, m]

        # =========== P2: scores -> masked exp -> E, c2 ===========
        with ExitStack() as pctx:
            stg2 = pctx.enter_context(tc.tile_pool(name="stg2", bufs=2))
            ps2 = pctx.enter_context(tc.tile_pool(name="ps2", bufs=4, space="PSUM"))
            for lt in range(NT_):
                c2p = stg2.tile([P, NMC], F32, tag="c2p")
                for mc in range(NMC):
                    sp = ps2.tile([P, NB], F32, tag="sp")
                    for ce in range(ND_):
                        nc.tensor.matmul(
                            sp[:], qT_sb[:, ce, lt * P:(lt + 1) * P],
                            kT_sb[:, ce, mc * NB:(mc + 1) * NB],
                            start=(ce == 0), stop=False)
                    nc.tensor.matmul(
                        sp[:], ones_row[:], negrow[0:1, mc * NB:(mc + 1) * NB],
                        start=False, stop=True)
                    nc.scalar.activation(
                        E_sb[:, lt, mc * NB:(mc + 1) * NB], sp[:],
                        mybir.ActivationFunctionType.Exp,
                        accum_out=c2p[:, mc:mc + 1])
                # c2 = sum of partials; c2r = 1/c2
                if NMC == 4:
                    a0 = stg2.tile([P, 1], F32, tag="c2a")
                    b0 = stg2.tile([P, 1], F32, tag="c2b")
                    c0 = stg2.tile([P, 1], F32, tag="c2c")
                    nc.vector.tensor_add(a0[:], c2p[:, 0:1], c2p[:, 1:2])
                    nc.vector.tensor_add(b0[:], c2p[:, 2:3], c2p[:, 3:4])
                    nc.vector.tensor_add(c0[:], a0[:], b0[:])
                elif NMC == 2:
                    c0 = stg2.tile([P, 1], F32, tag="c2c")
                    nc.vector.tensor_add(c0[:], c2p[:, 0:1], c2p[:, 1:2])
                else:
                    c0 = c2p[:, 0:1]
                    c0 = c2p
                    c0 = c2p[:, 0:1] if True else None
                nc.vector.reciprocal(c2r_sb[:, lt:lt + 1], c0[:] if NMC > 1 else c2p[:, 0:1])

        # =========== P3: x1_mid = maskcol/(c1+eps) * (E^T @ v1) ===========
        v1r = qk.tile([P, NT_, D_], BF, tag="qk")
        x1mid = qk.tile([P, NT_, D_], BF, tag="qk")
        with ExitStack() as pctx:
            stg3 = pctx.enter_context(tc.tile_pool(name="stg3", bufs=2))
            ps3 = pctx.enter_context(tc.tile_pool(name="ps3", bufs=4, space="PSUM"))
            ps3c = pctx.enter_context(tc.tile_pool(name="ps3c", bufs=2, space="PSUM"))
            for lt in range(NT_):
                nc.sync.dma_start(v1r[:, lt, :], v1_d[lt * P:(lt + 1) * P, :])
            for mt in range(NT_):
                mids = [ps3.tile([P, EH], F32, tag="mid", name=f"mid{e_}") for e_ in range(NEH)]
                c1p = ps3c.tile([P, 1], F32, tag="c1")
                for lc in range(NT_):
                    lhs = E_sb[:, lc, mt * P:(mt + 1) * P]
                    for eh in range(NEH):
                        nc.tensor.matmul(
                            mids[eh][:], lhs, v1r[:, lc, eh * EH:(eh + 1) * EH],
                            start=(lc == 0), stop=(lc == NT_ - 1))
                    nc.tensor.matmul(c1p[:], lhs, ones_col[:],
                                     start=(lc == 0), stop=(lc == NT_ - 1))
                c1e = stg3.tile([P, 1], F32, tag="c1e")
                c1r = stg3.tile([P, 1], F32, tag="c1r")
                r1 = stg3.tile([P, 1], F32, tag="r1")
                nc.vector.tensor_scalar_add(c1e[:], c1p[:], 1.0e-30)
                nc.vector.reciprocal(c1r[:], c1e[:])
                nc.vector.tensor_scalar_mul(r1[:], c1r[:], maskcol[:, mt:mt + 1])
                for eh in range(NEH):
                    nc.vector.tensor_scalar_mul(
                        x1mid[:, mt, eh * EH:(eh + 1) * EH], mids[eh][:], r1[:])

        # =========== P5: x2_out = EmT@v2 / c2 ; x1_out = EmT@x1mid / c2 =====
        with ExitStack() as pctx:
            stg5 = pctx.enter_context(tc.tile_pool(name="stg5", bufs=2))
            ps5 = pctx.enter_context(tc.tile_pool(name="ps5", bufs=8, space="PSUM"))
            for lt in range(NT_):
                etw = stg5.tile([P, NT_, P], BF, tag="etw")
                for j in range(NT_):
                    nc.sync.dma_start(
                        etw[:, j, :], E_sb[:, lt, j * P:(j + 1) * P],
                        transpose=True)
                o2 = [ps5.tile([P, EH], F32, tag="o", name=f"o2_{e_}") for e_ in range(NEH)]
                o1 = [ps5.tile([P, EH], F32, tag="o", name=f"o1_{e_}") for e_ in range(NEH)]
                for mc in range(NT_):
                    lhs = etw[:, mc, :]
                    st = (mc == 0)
                    sp_ = (mc == NT_ - 1)
                    for eh in range(NEH):
                        nc.tensor.matmul(
                            o2[eh][:], lhs, v2_sb[:, mc, eh * EH:(eh + 1) * EH],
                            start=st, stop=sp_)
                        nc.tensor.matmul(
                            o1[eh][:], lhs, x1mid[:, mc, eh * EH:(eh + 1) * EH],
                            start=st, stop=sp_)
                rec2 = c2r_sb[:, lt:lt + 1]
                o2stg = stg5.tile([P, D_], F32, tag="o2stg")
                o1stg = stg5.tile([P, D_], F32, tag="o1stg")
                for eh in range(NEH):
                    nc.vector.tensor_scalar_mul(
                        o2stg[:, eh * EH:(eh + 1) * EH], o2[eh][:], rec2)
                    nc.vector.tensor_scalar_mul(
                        o1stg[:, eh * EH:(eh + 1) * EH], o1[eh][:], rec2)
                nc.sync.dma_start(x2o_d[lt * P:(lt + 1) * P, :], o2stg[:])
                nc.sync.dma_start(x1o_d[lt * P:(lt + 1) * P, :], o1stg[:])

    nc.compile()
    return nc


def _get_nc():
    if "nc" not in _CACHE:
        _CACHE["nc"] = _build()
    return _CACHE["nc"]


def kernel(x1, x2, mask, Wq, bq, Wk, bk, Wv, bv):
    nc = _get_nc()
    from concourse.bass_utils import run_bass_kernel_spmd

    x1 = np.ascontiguousarray(np.asarray(x1, dtype=np.float32))
    x2 = np.ascontiguousarray(np.asarray(x2, dtype=np.float32))
    mask = np.ascontiguousarray(np.asarray(mask, dtype=np.float32))
    shared = {
        "Wq": np.ascontiguousarray(np.asarray(Wq, dtype=np.float32)),
        "bq": np.ascontiguousarray(np.asarray(bq, dtype=np.float32)),
        "Wk": np.ascontiguousarray(np.asarray(Wk, dtype=np.float32)),
        "bk": np.ascontiguousarray(np.asarray(bk, dtype=np.float32)),
        "Wv": np.ascontiguousarray(np.asarray(Wv, dtype=np.float32)),
        "bv": np.ascontiguousarray(np.asarray(bv, dtype=np.float32)),
    }
    in_maps = [
        {"x1": x1[c], "x2": x2[c], "mask": mask[c], **shared} for c in range(B)
    ]
    res = run_bass_kernel_spmd(nc, in_maps, core_ids=list(range(B)))
    x1_out = np.stack([res.results[c]["x1_out"] for c in range(B)])
    x2_out = np.stack([res.results[c]["x2_out"] for c in range(B)])
    return (x1_out, x2_out)
